# revision 1
# baseline (speedup 1.0000x reference)
"""Trainium2 Bass kernel for nn_BlockSelfAttentionModule.

Reference semantics (B=4, H=8, L=1024, I=16 instruments, F=64 frames, D=64):
  out[b*H+h, l, m] = q[l] . r_instrument[l%I, m%I, :, h]
                   + q[l] . a_h[(l//I - m//I) mod (F+1)]
  where a_h = concat(e_past[:, :, h], -111 pad row)   # (65, D)

Both bias terms factor through small per-row tables:
  Ui[l, c]  = q[l] . R_h[l%I, c]          (L x 16)
  Psh[l, f] = q[l] . a_h[(l//I - f) % 65] (L x 64)
  out[l, f*16 + c] = Psh[l, f] + Ui[l, c]

Strategy (8 cores, data parallel over the 32 = B*H rows, 4 rows/core):
  host: pre-transpose q to (d, l) layout; pre-gather the tiny tables so
        every in-kernel matmul rhs is a contiguous positive-stride slice
        (rt: R as (d, i*16+c); a2st: the reversed diagonal table, doubled
        and 1-shifted across two 64-partition halves so a K=128 matmul
        covers two frames at once).
  device, per row:
    - gpsimd/ACT build a block-diagonal zero-padded copy of qT (zq) so the
      PE can emit Psh for 2 frames per matmul with M=32 outputs, landing
      at the legal PSUM quadrant offsets {0,32,64,96}.
    - tiny fp32 PE matmuls produce Ui^T and the (128,64) Psh tile in
      PSUM; one PE transpose per 128-row tile re-orients Ui.
    - a single broadcast add expands (128,64)+(128,16) -> (128,1024).
    - DMA the 512 KiB tile out. The 16 MiB/core output write dominates
      (memory regime).
"""

import numpy as np

import concourse.bass as bass
import concourse.bacc as bacc
import concourse.mybir as mybir
from concourse import masks
from concourse.tile import TileContext
from concourse.bass_utils import run_bass_kernel_spmd

F32 = mybir.dt.float32

N_CORES = 8
ROWS_PER_CORE = 4  # (b*H + h) rows per core
L = 1024
D = 64
I = 16
F = 64
PAD_VAL = -111.0

_PROGRAM = None


def load_row_inputs(nc, pools, j, zq_eng="scalar"):
    """Issue the input DMAs + zq build for local row j; returns live tiles."""
    (qT, rt, a2st, out) = pools["dram"]
    (qpool, uitp, uitilep, outp) = pools["sbuf"]

    qt = qpool.tile([D, L], F32)
    nc.sync.dma_start(qt[:], qT[j])
    rtt = qpool.tile([D, I * I], F32)
    nc.sync.dma_start(rtt[:], rt[j])
    a2t = qpool.tile([128, 128], F32)
    nc.sync.dma_start(a2t[:], a2st[j])

    # zero-padded block-diagonal qT:
    # zq[s*64+d, l] = qT[d, l] if (l//16) % 2 == s else 0   (l = 32*fp+16*s+i)
    zq = qpool.tile([128, L], F32)
    nc.gpsimd.memset(zq[:], 0.0)
    qsplit = qt[:].rearrange("d (fp s i) -> d s fp i", s=2, i=I)
    zs0 = zq[0:64, :].rearrange("d (fp s i) -> d s fp i", s=2, i=I)
    zs1 = zq[64:128, :].rearrange("d (fp s i) -> d s fp i", s=2, i=I)
    if zq_eng == "scalar":
        nc.scalar.copy(zs0[:, 0, :], qsplit[:, 0, :])
        nc.scalar.copy(zs1[:, 1, :], qsplit[:, 1, :])
    else:
        nc.gpsimd.tensor_copy(zs0[:, 0, :], qsplit[:, 0, :])
        nc.gpsimd.tensor_copy(zs1[:, 1, :], qsplit[:, 1, :])
    return (qt, rtt, a2t, zq)


def build_row(nc, tc, pools, j, dma_all=True, gps_adds=0, zq_eng="scalar",
              loaded=None, split=1):
    """Emit the full pipeline for local row j (one b*H+h row)."""
    (qT, rt, a2st, out) = pools["dram"]
    (qpool, uitp, uitilep, outp) = pools["sbuf"]
    (ps_ui, ps_psh, ps_uit) = pools["psum"]
    ident = pools["ident"]

    if loaded is None:
        loaded = load_row_inputs(nc, pools, j, zq_eng)

    uiT = prep_uiT(nc, pools, loaded)
    for lt in range(8):  # 128-row tiles of l
        emit_tile(nc, pools, j, lt, loaded, uiT, dma_all, split)


def prep_uiT(nc, pools, loaded):
    """Ui^T[c, l] = q[l] . R_h[l%16, c], written l-ordered via a strided
    copy so the transpose input is a contiguous slice."""
    (qpool, uitp, uitilep, outp) = pools["sbuf"]
    (ps_ui, ps_psh, ps_uit) = pools["psum"]
    (qt, rtt, a2t, zq) = loaded
    # qt columns are l = f*16 + i; qv[:, i, :] selects one instrument
    qv = qt[:].rearrange("d (f i) -> d i f", i=I)
    uiT = uitp.tile([I, L], F32)
    uiTw = uiT[:].rearrange("c (f i) -> c i f", i=I)
    for i in range(I):
        ps = ps_ui.tile([I, F], F32)
        nc.tensor.matmul(ps[:], rtt[:, i * I : (i + 1) * I], qv[:, i, :])
        nc.scalar.copy(uiTw[:, i, :], ps[:])
    return uiT


def emit_tile(nc, pools, j, lt, loaded, uiT, dma_all=True, split=1):
    (qT, rt, a2st, out) = pools["dram"]
    (qpool, uitp, uitilep, outp) = pools["sbuf"]
    (ps_ui, ps_psh, ps_uit) = pools["psum"]
    ident = pools["ident"]
    (qt, rtt, a2t, zq) = loaded
    if True:
        psh = ps_psh.tile([128, F], F32)
        for g in range(4):
            fp = lt * 4 + g  # frame pair (2*fp, 2*fp+1)
            nc.tensor.matmul(
                psh[g * 32 : (g + 1) * 32, :],
                zq[:, fp * 32 : (fp + 1) * 32],
                a2t[:, 64 - 2 * fp : 128 - 2 * fp],
                tile_position=(0, g * 32),
            )

        upt = ps_uit.tile([128, I], F32)
        nc.tensor.transpose(upt[:], uiT[:, lt * 128 : (lt + 1) * 128], ident[:])
        uisb = uitilep.tile([128, I], F32)
        nc.scalar.copy(uisb[:], upt[:])

        ot = outp.tile([128, L], F32)
        dst = lt if dma_all else 0
        fh = F // split
        for h in range(split):
            ov = ot[:, h * fh * I : (h + 1) * fh * I].rearrange(
                "p (f c) -> p f c", c=I
            )
            in0 = (
                psh[:, h * fh : (h + 1) * fh]
                .unsqueeze(2)
                .broadcast_to([128, fh, I])
            )
            in1 = uisb[:].unsqueeze(1).broadcast_to([128, fh, I])
            nc.vector.scalar_tensor_tensor(
                ov, in0, 0.0, in1, mybir.AluOpType.bypass, mybir.AluOpType.add
            )
            nc.sync.dma_start(
                out[j, dst * 128 : (dst + 1) * 128,
                    h * fh * I : (h + 1) * fh * I],
                ot[:, h * fh * I : (h + 1) * fh * I],
            )


def build_program(loop_iters: int | None = None, out_bufs: int = 4,
                  psh_bufs: int = 2, gps_adds: int = 0,
                  zq_eng: str = "scalar", prefetch: bool = True,
                  staggered: bool = False, split: int = 1,
                  interleave: bool = False) -> bass.Bass:
    """loop_iters: when set, wrap the body in a device-side repeat loop
    (used only for benchmarking — amortizes host dispatch overhead)."""
    nc = bacc.Bacc("TRN2", debug=False, num_devices=N_CORES)
    qT = nc.declare_dram_parameter("qT", [ROWS_PER_CORE, D, L], F32, isOutput=False)
    rt = nc.declare_dram_parameter("rt", [ROWS_PER_CORE, D, I * I], F32, isOutput=False)
    a2st = nc.declare_dram_parameter("a2st", [ROWS_PER_CORE, 128, 128], F32, isOutput=False)
    out = nc.declare_dram_parameter("out", [ROWS_PER_CORE, L, L], F32, isOutput=True)

    with TileContext(nc) as tc:
        with (
            tc.tile_pool(name="const", bufs=1) as constp,
            tc.tile_pool(name="qpool", bufs=(4 if prefetch else 2)) as qpool,
            tc.tile_pool(name="uit", bufs=(4 if interleave else 2)) as uitp,
            tc.tile_pool(name="uitile", bufs=3) as uitilep,
            tc.tile_pool(name="outp", bufs=out_bufs) as outp,
            tc.tile_pool(name="ps_ui", bufs=2, space="PSUM") as ps_ui,
            tc.tile_pool(name="ps_psh", bufs=psh_bufs, space="PSUM") as ps_psh,
            tc.tile_pool(name="ps_uit", bufs=2, space="PSUM") as ps_uit,
        ):
            ident = constp.tile([I, I], F32)
            masks.make_identity(nc, ident[:])
            pools = {
                "dram": (qT, rt, a2st, out),
                "sbuf": (qpool, uitp, uitilep, outp),
                "psum": (ps_ui, ps_psh, ps_uit),
                "ident": ident,
            }

            def body(_iv=None):
                if interleave:
                    loaded = [load_row_inputs(nc, pools, j, zq_eng)
                              for j in range(ROWS_PER_CORE)]
                    uiTs = [prep_uiT(nc, pools, loaded[j])
                            for j in range(ROWS_PER_CORE)]
                    for lt in range(8):
                        for j in range(ROWS_PER_CORE):
                            emit_tile(nc, pools, j, lt, loaded[j], uiTs[j],
                                      True, split)
                    return
                if prefetch:
                    loaded = [
                        load_row_inputs(nc, pools, j, zq_eng)
                        for j in range(ROWS_PER_CORE)
                    ]
                else:
                    loaded = [None] * ROWS_PER_CORE
                for j in range(ROWS_PER_CORE):
                    build_row(nc, tc, pools, j, gps_adds=gps_adds,
                              zq_eng=zq_eng, loaded=loaded[j], split=split)

            if loop_iters is None:
                body()
            else:
                with tc.For_i(0, loop_iters, 1,
                              staggered_reset=staggered) as _iv:
                    body(_iv)
    return nc


def make_in_maps(q, r_instrument, e_past):
    """Host-side sharding + table prep. Returns per-core input dicts."""
    q = np.asarray(q, dtype=np.float32)
    r_instrument = np.asarray(r_instrument, dtype=np.float32)
    e_past = np.asarray(e_past, dtype=np.float32)

    qc = q.reshape(N_CORES, ROWS_PER_CORE, L, D).transpose(0, 1, 3, 2)

    a = np.concatenate(
        [e_past, np.full((1, D, 8), PAD_VAL, dtype=np.float32)], axis=0
    )  # (65, D, H)
    # a2st_h[s*64+d, t] = a_h[(64 - t + s) % 65, d]
    idx2 = (64 - np.arange(128)[None, :] + np.arange(2)[:, None]) % 65  # (2, 128)
    a2st_all = a[idx2]  # (2, 128, 64, 8)
    a2st_all = a2st_all.transpose(3, 0, 2, 1).reshape(8, 128, 128)

    rt_all = r_instrument.transpose(3, 2, 0, 1).reshape(8, D, I * I)

    in_maps = []
    for k in range(N_CORES):
        hs = [(ROWS_PER_CORE * k + j) % 8 for j in range(ROWS_PER_CORE)]
        in_maps.append(
            {
                "qT": np.ascontiguousarray(qc[k]),
                "rt": np.ascontiguousarray(rt_all[hs]),
                "a2st": np.ascontiguousarray(a2st_all[hs]),
            }
        )
    return in_maps


def _get_program() -> bass.Bass:
    global _PROGRAM
    if _PROGRAM is None:
        _PROGRAM = build_program()
        if not _PROGRAM.is_finalized():
            _PROGRAM.finalize()
    return _PROGRAM


def kernel(q, r_instrument, e_past, flipped_masks=None, **_unused):
    in_maps = make_in_maps(q, r_instrument, e_past)
    res = run_bass_kernel_spmd(_get_program(), in_maps, list(range(N_CORES))).results
    out = np.concatenate([res[k]["out"] for k in range(N_CORES)], axis=0)
    return out.reshape(N_CORES * ROWS_PER_CORE, L, L)

